# revision 26
# baseline (speedup 1.0000x reference)
"""Causal multi-head attention (B=4, T=2048, C=1024, H=16) on 8 TRN2 cores.

Sharding: core c = (batch b = c // 2, head-group g = c % 2). Each core runs
QKV projection + attention + its half of the output projection for one
batch and 8 heads (Megatron-style column-parallel wqkv / row-parallel wo);
the two partial wo products per batch are summed on the host during
unsharding.

Device layouts (per core):
  xT    [C, T]        x[b] transposed (C on partitions for QKV matmuls)
  wqkvT [C, 3*512]    head-group slice of wqkv, transposed; cols [Q|K|V]
  woT   [512, C]      wo columns for this head-group, transposed
  outT  [C, T]        partial (x @ wqkv.T -> attn -> @ wo.T).T

All matmuls run in float32r (tf32-like: ~1.5e-4 rel err, ~3.3x faster
than fp32 on the PE). Scores are computed transposed (k on partitions) so
softmax P.T feeds the AV matmul directly with no transposes; the softmax
denominator rides along as a 65th row of V (column of ones), and the
causal mask is an affine_select on the diagonal score tiles. exp() is
applied without max-subtraction: scores are ~N(0,1), far inside fp32 exp
range.

Scheduling: the T dimension is processed in four 512-chunks. Attention on
chunk ch is emitted with the (PE-only) QKV projection of chunk ch+1 and
the WO projection of chunk ch-1 round-robin-interleaved into its
(head-pair, k-tile) loop, so the PE array keeps streaming while ACT runs
the softmax exps — that keeps the HAM clock gate at 2.4 GHz. Heads are
processed in pairs living at partition offsets 0/64: their K=64 score
matmuls target disjoint row-halves of the PE array and run concurrently.
"""

import sys

sys.path.insert(0, "/opt/trn_rl_repo")

import numpy as np

import concourse.bass as bass  # noqa: F401  (AP helpers)
import concourse.mybir as mybir
import concourse.tile as tile
from concourse import bacc
from concourse.bass_utils import run_bass_kernel_spmd

F32 = mybir.dt.float32
F32R = mybir.dt.float32r
EXP = mybir.ActivationFunctionType.Exp

B, T, C, H = 4, 2048, 1024, 16
HD = 64  # head dim
HPC = 8  # heads per core
GF = HPC * HD  # 512 group features
CHUNK = 512
NCHUNK = T // CHUNK  # 4
KT_C = C // 128  # 8 k-tiles over C
SCALE = 1.0 / np.sqrt(HD)

_CACHE = {}


def _build():
    nc = bacc.Bacc(
        "TRN2", target_bir_lowering=False, debug=False, num_devices=8
    )
    xT = nc.dram_tensor("xT", [C, T], F32R, kind="ExternalInput")
    wqkvT = nc.dram_tensor("wqkvT", [C, 3 * GF], F32R, kind="ExternalInput")
    woT = nc.dram_tensor("woT", [GF, C], F32R, kind="ExternalInput")
    outT = nc.dram_tensor("outT", [C, T], F32, kind="ExternalOutput")

    xT_re = xT.ap().rearrange("(kt p) t -> p kt t", p=128)
    wq_re = wqkvT.ap().rearrange("(kt p) f -> p kt f", p=128)
    wo_re = woT.ap().rearrange("(kt p) f -> p kt f", p=128)

    with tile.TileContext(nc) as tc:
        with (
            tc.tile_pool(name="weights", bufs=1) as wp,
            tc.tile_pool(name="persist", bufs=1) as persist,
            tc.tile_pool(name="xin", bufs=2) as xp,
            tc.tile_pool(name="qt", bufs=2) as qtp,
            tc.tile_pool(name="pt", bufs=2) as ptp,
            tc.tile_pool(name="yt", bufs=3) as ytp,
            tc.tile_pool(name="small", bufs=2) as smp,
            tc.tile_pool(name="ost", bufs=2) as ostp,
            tc.tile_pool(name="proj", bufs=1, space="PSUM") as projp,
            tc.tile_pool(name="sps", bufs=2, space="PSUM") as spsp,
            tc.tile_pool(name="pops", bufs=3, space="PSUM") as pop,
        ):
            # wq split column-wise: [Q four 128-col tiles][K four][V one 512-col]
            wqq = [[wp.tile([128, 128], F32R, name=f"wqq{kt}_{g}") for g in range(4)]
                   for kt in range(KT_C)]
            wqk = [[wp.tile([128, 128], F32R, name=f"wqk{kt}_{g}") for g in range(4)]
                   for kt in range(KT_C)]
            wqv = [wp.tile([128, GF], F32R, name=f"wqv{kt}") for kt in range(KT_C)]
            wo_sb = wp.tile([128, GF // 128, C], F32R)

            ones32 = persist.tile([128, GF // 128 * HPC], F32)
            nc.vector.memset(ones32[:], 1.0)

            # per-chunk persistent K.T / V_aug
            KTc = [persist.tile([128, 4, CHUNK], F32R, name=f"KT{i}") for i in range(NCHUNK)]
            VAc = [persist.tile([128, 4, HPC, HD + 1], F32R, name=f"VA{i}") for i in range(NCHUNK)]
            for i in range(NCHUNK):
                nc.vector.tensor_copy(
                    VAc[i][:, :, :, HD : HD + 1],
                    ones32[:].rearrange("p (t h) -> p t h", t=4)[:, :, :, None],
                )

            xts = {}
            QTcs = {}
            ytcs = {}

            def load_x(ch):
                xts[ch] = [
                    xp.tile([128, CHUNK], F32R, tag="xt", bufs=11, name=f"xt{ch}_{kt}")
                    for kt in range(KT_C)
                ]
                engs = (
                    [nc.sync, nc.gpsimd, nc.scalar] if ch == 0 else [nc.sync]
                )
                for kt in range(KT_C):
                    engs[kt % len(engs)].dma_start(
                        xts[ch][kt][:],
                        xT_re[:, kt, ch * CHUNK : (ch + 1) * CHUNK],
                    )

            def qkv_thunks(ch, spread=False):
                """Projection chains for chunk ch (12 thunks)."""
                QTcs[ch] = qtp.tile([128, 4, CHUNK], F32R, tag="qtc", name=f"qtc{ch}")
                thunks = []
                seq = [0]

                def chain_psum():
                    if not spread:
                        return projp.tile([128, CHUNK], F32, tag="proj", name="ps")
                    i = seq[0]
                    seq[0] += 1
                    pool, tag = [
                        (projp, "proj"), (pop, "po"), (pop, "po"), (pop, "po"),
                        (spsp, "sps"), (spsp, "sps"),
                    ][i % 6]
                    return pool.tile([128, CHUNK], F32, tag=tag, name=f"pps{ch}_{i}")

                def qk_chain(m):
                    xt = xts[ch]
                    ps = chain_psum()
                    w = wqq[0][0]  # placeholder for loop binding
                    for kt in range(KT_C):
                        w = wqq[kt][m] if m < 4 else wqk[kt][m - 4]
                        nc.tensor.matmul(
                            ps[:],
                            w[:],
                            xt[kt][:],
                            start=(kt == 0),
                            stop=(kt == KT_C - 1),
                        )
                    dst = QTcs[ch][:, m] if m < 4 else KTc[ch][:, m - 4]
                    nc.vector.tensor_copy(dst, ps[:])

                def v_chain(t4):
                    xt = xts[ch]
                    ps = chain_psum()
                    for kt in range(KT_C):
                        nc.tensor.matmul(
                            ps[:],
                            xt[kt][:, t4 * 128 : (t4 + 1) * 128],
                            wqv[kt][:],
                            start=(kt == 0),
                            stop=(kt == KT_C - 1),
                        )
                    nc.vector.tensor_copy(
                        VAc[ch][:, t4, :, 0:HD],
                        ps[:].rearrange("p (h d) -> p h d", h=HPC),
                    )

                thunks.append(lambda: qk_chain(0))
                thunks.append(lambda: qk_chain(4))
                for t4 in range(4):
                    thunks.append(lambda t4=t4: v_chain(t4))
                for m in (1, 5, 2, 6, 3, 7):
                    thunks.append(lambda m=m: qk_chain(m))
                return thunks

            def wo_thunks(ch, pool=None):
                """Output projection chains for chunk ch (8 thunks)."""
                cs = slice(ch * CHUNK, (ch + 1) * CHUNK)
                wo_pool = pool or projp

                def wo_chain(m):
                    ytc = ytcs[ch]
                    wps = wo_pool.tile([128, CHUNK], F32, tag="proj" if wo_pool is projp else "sps", name=f"wop{ch}_{m}")
                    for kt in range(GF // 128):
                        nc.tensor.matmul(
                            wps[:],
                            wo_sb[:, kt, m * 128 : (m + 1) * 128],
                            ytc[:, kt],
                            start=(kt == 0),
                            stop=(kt == GF // 128 - 1),
                        )
                    ot = ostp.tile([128, CHUNK], F32)
                    nc.vector.tensor_copy(ot[:], wps[:])
                    nc.sync.dma_start(outT[m * 128 : (m + 1) * 128, cs], ot[:])

                return [lambda m=m: wo_chain(m) for m in range(8)]

            def attention(qc, fill):
                """Attention for q-chunk qc, popping `fill` thunks along the way."""
                ytcs[qc] = ytp.tile([128, GF // 128, CHUNK], F32R, tag="ytc", name=f"ytc{qc}")
                ytc = ytcs[qc]
                QTc = QTcs[qc]
                nkt = (qc + 1) * 4
                nsteps = 4 * nkt
                stride = max(1, -(-nsteps // max(1, len(fill))))
                step = 0
                for pair in range(4):  # heads (2*pair, 2*pair+1)
                    pos = [
                        pop.tile([65, CHUNK], F32, tag="po", name=f"po{s}")
                        for s in range(2)
                    ]
                    for kt in range(nkt):
                        kc, lk = kt // 4, kt % 4
                        j = kt - 4 * qc
                        w = CHUNK if j < 0 else CHUNK - j * 128
                        q0 = CHUNK - w
                        sps2 = spsp.tile([128, 2 * CHUNK], F32, tag="sps")
                        for s in range(2):  # the two heads of the pair
                            off = s * 64
                            nc.tensor.matmul(
                                sps2[:, s * CHUNK : s * CHUNK + w],
                                KTc[kc][off : off + 64, pair, lk * 128 : (lk + 1) * 128],
                                QTc[off : off + 64, pair, q0:CHUNK],
                                start=True,
                                stop=True,
                            )
                        pt = ptp.tile([128, 2 * CHUNK], F32R)
                        nc.scalar.activation(
                            pt[:, 0 : CHUNK + w],
                            sps2[:, 0 : CHUNK + w],
                            EXP,
                            scale=SCALE,
                        )
                        if j >= 0:
                            for s in range(2):
                                nc.gpsimd.affine_select(
                                    out=pt[:, s * CHUNK : s * CHUNK + w],
                                    in_=pt[:, s * CHUNK : s * CHUNK + w],
                                    compare_op=mybir.AluOpType.is_ge,
                                    fill=0.0,
                                    base=0,
                                    pattern=[[1, w]],
                                    channel_multiplier=-1,
                                )
                        for s in range(2):
                            h = 2 * pair + s
                            nc.tensor.matmul(
                                pos[s][:, q0:CHUNK],
                                VAc[kc][:, lk, h],
                                pt[:, s * CHUNK : s * CHUNK + w],
                                start=(kt == 0),
                                stop=(kt == nkt - 1),
                            )
                        step += 1
                        if fill and step % stride == 0:
                            fill.pop(0)()
                    for s in range(2):
                        h = 2 * pair + s
                        off = s * 64
                        po = pos[s]
                        dn = smp.tile([1, CHUNK], F32, tag="nrm", bufs=2, name="dn")
                        nc.vector.tensor_copy(dn[:], po[64:65, :])
                        rc = smp.tile([1, CHUNK], F32, tag="nrm", bufs=2, name="rc")
                        nc.vector.reciprocal_approx_fast(rc[:], dn[:])
                        bc = smp.tile([64, CHUNK], F32, tag="nrm", bufs=2, name="bc")
                        nc.gpsimd.partition_broadcast(bc[:], rc[:])
                        nc.vector.tensor_mul(
                            ytc[off : off + 64, pair, :], po[0:64, :], bc[:]
                        )
                while fill:
                    fill.pop(0)()

            # prologue: x(0) first so QKV(0) starts ASAP; wo last (needed
            # only from the ch=2 window on)
            load_x(0)
            _dma_engs = [nc.gpsimd, nc.scalar, nc.sync]
            _dq = [0]

            def _dma(dst, srcslice):
                _dma_engs[_dq[0] % 3].dma_start(dst, srcslice)
                _dq[0] += 1

            # pair-0-critical first: Q col-group 0, K col-group 0, V block
            for kt in range(KT_C):
                _dma(wqq[kt][0][:], wq_re[:, kt, 0:128])
                _dma(wqk[kt][0][:], wq_re[:, kt, GF : GF + 128])
            for kt in range(KT_C):
                _dma(wqv[kt][:], wq_re[:, kt, 2 * GF : 3 * GF])
            for g in (1, 2, 3):
                for kt in range(KT_C):
                    _dma(wqq[kt][g][:], wq_re[:, kt, g * 128 : (g + 1) * 128])
                    _dma(wqk[kt][g][:], wq_re[:, kt, GF + g * 128 : GF + (g + 1) * 128])
            for kt in range(GF // 128):
                _dma(wo_sb[:, kt], wo_re[:, kt])
            for t in qkv_thunks(0, spread=True):
                t()
            for ch in range(NCHUNK):
                if ch + 1 < NCHUNK:
                    load_x(ch + 1)
                fill = []
                if ch + 1 < NCHUNK:
                    fill += qkv_thunks(ch + 1)
                if ch == NCHUNK - 1:
                    fill += wo_thunks(1) + wo_thunks(2)
                elif ch - 2 >= 0:
                    fill += wo_thunks(ch - 2)
                attention(ch, fill)
            for t in wo_thunks(NCHUNK - 1, pool=spsp):
                t()

    nc.compile()
    return nc


def _prep_inputs(x, wqkv, wo):
    """Per-core input maps: core c = (batch c // 2, head-group c % 2)."""
    x = np.asarray(x, dtype=np.float32)
    wqkv = np.asarray(wqkv, dtype=np.float32)
    wo = np.asarray(wo, dtype=np.float32)
    in_maps = []
    for c in range(8):
        b, g = c // 2, c % 2
        rows = np.r_[
            g * GF : (g + 1) * GF,
            C + g * GF : C + (g + 1) * GF,
            2 * C + g * GF : 2 * C + (g + 1) * GF,
        ]
        in_maps.append(
            {
                "xT": np.ascontiguousarray(x[b].T),
                "wqkvT": np.ascontiguousarray(wqkv[rows].T),
                "woT": np.ascontiguousarray(wo[:, g * GF : (g + 1) * GF].T),
            }
        )
    return in_maps


def _run(x, wqkv, wo, trace=False, trace_cores=None):
    if "nc" not in _CACHE:
        _CACHE["nc"] = _build()
    res = run_bass_kernel_spmd(
        _CACHE["nc"],
        _prep_inputs(x, wqkv, wo),
        core_ids=list(range(8)),
        trace=trace,
        trace_cores=trace_cores,
    )
    out = np.empty((B, T, C), dtype=np.float32)
    for b in range(B):
        out[b] = (res.results[2 * b]["outT"] + res.results[2 * b + 1]["outT"]).T
    return out, res


def kernel(x, wqkv, wo):
    out, _ = _run(x, wqkv, wo)
    return out


# revision 27
# speedup vs baseline: 1.0028x; 1.0028x over previous
"""Causal multi-head attention (B=4, T=2048, C=1024, H=16) on 8 TRN2 cores.

Sharding: core c = (batch b = c // 2, head-group g = c % 2). Each core runs
QKV projection + attention + its half of the output projection for one
batch and 8 heads (Megatron-style column-parallel wqkv / row-parallel wo);
the two partial wo products per batch are summed on the host during
unsharding.

Device layouts (per core):
  xT    [C, T]        x[b] transposed (C on partitions for QKV matmuls)
  wqkvT [C, 3*512]    head-group slice of wqkv, transposed; cols [Q|K|V]
  woT   [512, C]      wo columns for this head-group, transposed
  outT  [C, T]        partial (x @ wqkv.T -> attn -> @ wo.T).T

All matmuls run in float32r (tf32-like: ~1.5e-4 rel err, ~3.3x faster
than fp32 on the PE). Scores are computed transposed (k on partitions) so
softmax P.T feeds the AV matmul directly with no transposes; the softmax
denominator rides along as a 65th row of V (column of ones), and the
causal mask is an affine_select on the diagonal score tiles. exp() is
applied without max-subtraction: scores are ~N(0,1), far inside fp32 exp
range.

Scheduling: the T dimension is processed in four 512-chunks. Attention on
chunk ch is emitted with the (PE-only) QKV projection of chunk ch+1 and
the WO projection of chunk ch-1 round-robin-interleaved into its
(head-pair, k-tile) loop, so the PE array keeps streaming while ACT runs
the softmax exps — that keeps the HAM clock gate at 2.4 GHz. Heads are
processed in pairs living at partition offsets 0/64: their K=64 score
matmuls target disjoint row-halves of the PE array and run concurrently.
"""

import sys

sys.path.insert(0, "/opt/trn_rl_repo")

import numpy as np

import concourse.bass as bass  # noqa: F401  (AP helpers)
import concourse.mybir as mybir
import concourse.tile as tile
from concourse import bacc
from concourse.bass_utils import run_bass_kernel_spmd

F32 = mybir.dt.float32
F32R = mybir.dt.float32r
EXP = mybir.ActivationFunctionType.Exp

B, T, C, H = 4, 2048, 1024, 16
HD = 64  # head dim
HPC = 8  # heads per core
GF = HPC * HD  # 512 group features
CHUNK = 512
NCHUNK = T // CHUNK  # 4
KT_C = C // 128  # 8 k-tiles over C
SCALE = 1.0 / np.sqrt(HD)

_CACHE = {}


def _build():
    nc = bacc.Bacc(
        "TRN2", target_bir_lowering=False, debug=False, num_devices=8
    )
    xT = nc.dram_tensor("xT", [C, T], F32R, kind="ExternalInput")
    wqkvT = nc.dram_tensor("wqkvT", [C, 3 * GF], F32R, kind="ExternalInput")
    woT = nc.dram_tensor("woT", [GF, C], F32R, kind="ExternalInput")
    outT = nc.dram_tensor("outT", [C, T], F32, kind="ExternalOutput")

    xT_re = xT.ap().rearrange("(kt p) t -> p kt t", p=128)
    wq_re = wqkvT.ap().rearrange("(kt p) f -> p kt f", p=128)
    wo_re = woT.ap().rearrange("(kt p) f -> p kt f", p=128)

    with tile.TileContext(nc) as tc:
        with (
            tc.tile_pool(name="weights", bufs=1) as wp,
            tc.tile_pool(name="persist", bufs=1) as persist,
            tc.tile_pool(name="xin", bufs=2) as xp,
            tc.tile_pool(name="qt", bufs=2) as qtp,
            tc.tile_pool(name="pt", bufs=2) as ptp,
            tc.tile_pool(name="yt", bufs=3) as ytp,
            tc.tile_pool(name="small", bufs=2) as smp,
            tc.tile_pool(name="ost", bufs=2) as ostp,
            tc.tile_pool(name="proj", bufs=1, space="PSUM") as projp,
            tc.tile_pool(name="sps", bufs=2, space="PSUM") as spsp,
            tc.tile_pool(name="pops", bufs=3, space="PSUM") as pop,
        ):
            # wq split column-wise: [Q four 128-col tiles][K four][V one 512-col]
            wqq = [[wp.tile([128, 128], F32R, name=f"wqq{kt}_{g}") for g in range(4)]
                   for kt in range(KT_C)]
            wqk = [[wp.tile([128, 128], F32R, name=f"wqk{kt}_{g}") for g in range(4)]
                   for kt in range(KT_C)]
            wqv = [wp.tile([128, GF], F32R, name=f"wqv{kt}") for kt in range(KT_C)]
            wo_sb = wp.tile([128, GF // 128, C], F32R)

            ones32 = persist.tile([128, GF // 128 * HPC], F32)
            nc.vector.memset(ones32[:], 1.0)

            # per-chunk persistent K.T / V_aug
            KTc = [persist.tile([128, 4, CHUNK], F32R, name=f"KT{i}") for i in range(NCHUNK)]
            VAc = [persist.tile([128, 4, HPC, HD + 1], F32R, name=f"VA{i}") for i in range(NCHUNK)]
            for i in range(NCHUNK):
                nc.vector.tensor_copy(
                    VAc[i][:, :, :, HD : HD + 1],
                    ones32[:].rearrange("p (t h) -> p t h", t=4)[:, :, :, None],
                )

            xts = {}
            QTcs = {}
            ytcs = {}

            def load_x(ch):
                xts[ch] = [
                    xp.tile([128, CHUNK], F32R, tag="xt", bufs=11, name=f"xt{ch}_{kt}")
                    for kt in range(KT_C)
                ]
                for kt in range(KT_C):
                    nc.sync.dma_start(
                        xts[ch][kt][:],
                        xT_re[:, kt, ch * CHUNK : (ch + 1) * CHUNK],
                    )

            def qkv_thunks(ch, spread=False):
                """Projection chains for chunk ch (12 thunks)."""
                QTcs[ch] = qtp.tile([128, 4, CHUNK], F32R, tag="qtc", name=f"qtc{ch}")
                thunks = []
                seq = [0]

                def chain_psum():
                    if not spread:
                        return projp.tile([128, CHUNK], F32, tag="proj", name="ps")
                    i = seq[0]
                    seq[0] += 1
                    pool, tag = [
                        (projp, "proj"), (pop, "po"), (pop, "po"), (pop, "po"),
                        (spsp, "sps"), (spsp, "sps"),
                    ][i % 6]
                    return pool.tile([128, CHUNK], F32, tag=tag, name=f"pps{ch}_{i}")

                def qk_chain(m):
                    xt = xts[ch]
                    ps = chain_psum()
                    w = wqq[0][0]  # placeholder for loop binding
                    for kt in range(KT_C):
                        w = wqq[kt][m] if m < 4 else wqk[kt][m - 4]
                        nc.tensor.matmul(
                            ps[:],
                            w[:],
                            xt[kt][:],
                            start=(kt == 0),
                            stop=(kt == KT_C - 1),
                        )
                    dst = QTcs[ch][:, m] if m < 4 else KTc[ch][:, m - 4]
                    nc.vector.tensor_copy(dst, ps[:])

                def v_chain(t4):
                    xt = xts[ch]
                    ps = chain_psum()
                    for kt in range(KT_C):
                        nc.tensor.matmul(
                            ps[:],
                            xt[kt][:, t4 * 128 : (t4 + 1) * 128],
                            wqv[kt][:],
                            start=(kt == 0),
                            stop=(kt == KT_C - 1),
                        )
                    nc.vector.tensor_copy(
                        VAc[ch][:, t4, :, 0:HD],
                        ps[:].rearrange("p (h d) -> p h d", h=HPC),
                    )

                thunks.append(lambda: qk_chain(0))
                thunks.append(lambda: qk_chain(4))
                for t4 in range(4):
                    thunks.append(lambda t4=t4: v_chain(t4))
                for m in (1, 5, 2, 6, 3, 7):
                    thunks.append(lambda m=m: qk_chain(m))
                return thunks

            def wo_thunks(ch, pool=None):
                """Output projection chains for chunk ch (8 thunks)."""
                cs = slice(ch * CHUNK, (ch + 1) * CHUNK)
                wo_pool = pool or projp

                def wo_chain(m):
                    ytc = ytcs[ch]
                    wps = wo_pool.tile([128, CHUNK], F32, tag="proj" if wo_pool is projp else "sps", name=f"wop{ch}_{m}")
                    for kt in range(GF // 128):
                        nc.tensor.matmul(
                            wps[:],
                            wo_sb[:, kt, m * 128 : (m + 1) * 128],
                            ytc[:, kt],
                            start=(kt == 0),
                            stop=(kt == GF // 128 - 1),
                        )
                    ot = ostp.tile([128, CHUNK], F32)
                    nc.vector.tensor_copy(ot[:], wps[:])
                    nc.sync.dma_start(outT[m * 128 : (m + 1) * 128, cs], ot[:])

                return [lambda m=m: wo_chain(m) for m in range(8)]

            def attention(qc, fill):
                """Attention for q-chunk qc, popping `fill` thunks along the way."""
                ytcs[qc] = ytp.tile([128, GF // 128, CHUNK], F32R, tag="ytc", name=f"ytc{qc}")
                ytc = ytcs[qc]
                QTc = QTcs[qc]
                nkt = (qc + 1) * 4
                nsteps = 4 * nkt
                stride = max(1, -(-nsteps // max(1, len(fill))))
                step = 0
                for pair in range(4):  # heads (2*pair, 2*pair+1)
                    pos = [
                        pop.tile([65, CHUNK], F32, tag="po", name=f"po{s}")
                        for s in range(2)
                    ]
                    for kt in range(nkt):
                        kc, lk = kt // 4, kt % 4
                        j = kt - 4 * qc
                        w = CHUNK if j < 0 else CHUNK - j * 128
                        q0 = CHUNK - w
                        sps2 = spsp.tile([128, 2 * CHUNK], F32, tag="sps")
                        for s in range(2):  # the two heads of the pair
                            off = s * 64
                            nc.tensor.matmul(
                                sps2[:, s * CHUNK : s * CHUNK + w],
                                KTc[kc][off : off + 64, pair, lk * 128 : (lk + 1) * 128],
                                QTc[off : off + 64, pair, q0:CHUNK],
                                start=True,
                                stop=True,
                            )
                        pt = ptp.tile([128, 2 * CHUNK], F32R)
                        nc.scalar.activation(
                            pt[:, 0 : CHUNK + w],
                            sps2[:, 0 : CHUNK + w],
                            EXP,
                            scale=SCALE,
                        )
                        if j >= 0:
                            for s in range(2):
                                nc.gpsimd.affine_select(
                                    out=pt[:, s * CHUNK : s * CHUNK + w],
                                    in_=pt[:, s * CHUNK : s * CHUNK + w],
                                    compare_op=mybir.AluOpType.is_ge,
                                    fill=0.0,
                                    base=0,
                                    pattern=[[1, w]],
                                    channel_multiplier=-1,
                                )
                        for s in range(2):
                            h = 2 * pair + s
                            nc.tensor.matmul(
                                pos[s][:, q0:CHUNK],
                                VAc[kc][:, lk, h],
                                pt[:, s * CHUNK : s * CHUNK + w],
                                start=(kt == 0),
                                stop=(kt == nkt - 1),
                            )
                        step += 1
                        if fill and step % stride == 0:
                            fill.pop(0)()
                    for s in range(2):
                        h = 2 * pair + s
                        off = s * 64
                        po = pos[s]
                        dn = smp.tile([1, CHUNK], F32, tag="nrm", bufs=2, name="dn")
                        nc.vector.tensor_copy(dn[:], po[64:65, :])
                        rc = smp.tile([1, CHUNK], F32, tag="nrm", bufs=2, name="rc")
                        nc.vector.reciprocal_approx_fast(rc[:], dn[:])
                        bc = smp.tile([64, CHUNK], F32, tag="nrm", bufs=2, name="bc")
                        nc.gpsimd.partition_broadcast(bc[:], rc[:])
                        nc.vector.tensor_mul(
                            ytc[off : off + 64, pair, :], po[0:64, :], bc[:]
                        )
                while fill:
                    fill.pop(0)()

            # prologue: x(0) first so QKV(0) starts ASAP; wo last (needed
            # only from the ch=2 window on)
            load_x(0)
            _dma_engs = [nc.gpsimd, nc.scalar, nc.sync]
            _dq = [0]

            def _dma(dst, srcslice):
                _dma_engs[_dq[0] % 3].dma_start(dst, srcslice)
                _dq[0] += 1

            # pair-0-critical first: Q col-group 0, K col-group 0, V block
            for kt in range(KT_C):
                _dma(wqq[kt][0][:], wq_re[:, kt, 0:128])
                _dma(wqk[kt][0][:], wq_re[:, kt, GF : GF + 128])
            for kt in range(KT_C):
                _dma(wqv[kt][:], wq_re[:, kt, 2 * GF : 3 * GF])
            for g in (1, 2, 3):
                for kt in range(KT_C):
                    _dma(wqq[kt][g][:], wq_re[:, kt, g * 128 : (g + 1) * 128])
                    _dma(wqk[kt][g][:], wq_re[:, kt, GF + g * 128 : GF + (g + 1) * 128])
            for kt in range(GF // 128):
                _dma(wo_sb[:, kt], wo_re[:, kt])
            for t in qkv_thunks(0, spread=True):
                t()
            for ch in range(NCHUNK):
                if ch + 1 < NCHUNK:
                    load_x(ch + 1)
                fill = []
                if ch + 1 < NCHUNK:
                    fill += qkv_thunks(ch + 1)
                if ch == NCHUNK - 1:
                    fill += wo_thunks(1) + wo_thunks(2)
                elif ch - 2 >= 0:
                    fill += wo_thunks(ch - 2)
                attention(ch, fill)
            for t in wo_thunks(NCHUNK - 1, pool=spsp):
                t()

    nc.compile()
    return nc


def _prep_inputs(x, wqkv, wo):
    """Per-core input maps: core c = (batch c // 2, head-group c % 2)."""
    x = np.asarray(x, dtype=np.float32)
    wqkv = np.asarray(wqkv, dtype=np.float32)
    wo = np.asarray(wo, dtype=np.float32)
    in_maps = []
    for c in range(8):
        b, g = c // 2, c % 2
        rows = np.r_[
            g * GF : (g + 1) * GF,
            C + g * GF : C + (g + 1) * GF,
            2 * C + g * GF : 2 * C + (g + 1) * GF,
        ]
        in_maps.append(
            {
                "xT": np.ascontiguousarray(x[b].T),
                "wqkvT": np.ascontiguousarray(wqkv[rows].T),
                "woT": np.ascontiguousarray(wo[:, g * GF : (g + 1) * GF].T),
            }
        )
    return in_maps


def _run(x, wqkv, wo, trace=False, trace_cores=None):
    if "nc" not in _CACHE:
        _CACHE["nc"] = _build()
    res = run_bass_kernel_spmd(
        _CACHE["nc"],
        _prep_inputs(x, wqkv, wo),
        core_ids=list(range(8)),
        trace=trace,
        trace_cores=trace_cores,
    )
    out = np.empty((B, T, C), dtype=np.float32)
    for b in range(B):
        out[b] = (res.results[2 * b]["outT"] + res.results[2 * b + 1]["outT"]).T
    return out, res


def kernel(x, wqkv, wo):
    out, _ = _run(x, wqkv, wo)
    return out


# revision 28
# speedup vs baseline: 1.0057x; 1.0029x over previous
"""Causal multi-head attention (B=4, T=2048, C=1024, H=16) on 8 TRN2 cores.

Sharding: core c = (batch b = c // 2, head-group g = c % 2). Each core runs
QKV projection + attention + its half of the output projection for one
batch and 8 heads (Megatron-style column-parallel wqkv / row-parallel wo);
the two partial wo products per batch are summed on the host during
unsharding.

Device layouts (per core):
  xT    [C, T]        x[b] transposed (C on partitions for QKV matmuls)
  wqkvT [C, 3*512]    head-group slice of wqkv, transposed; cols [Q|K|V]
  woT   [512, C]      wo columns for this head-group, transposed
  outT  [C, T]        partial (x @ wqkv.T -> attn -> @ wo.T).T

All matmuls run in float32r (tf32-like: ~1.5e-4 rel err, ~3.3x faster
than fp32 on the PE). Scores are computed transposed (k on partitions) so
softmax P.T feeds the AV matmul directly with no transposes; the softmax
denominator rides along as a 65th row of V (column of ones), and the
causal mask is an affine_select on the diagonal score tiles. exp() is
applied without max-subtraction: scores are ~N(0,1), far inside fp32 exp
range.

Scheduling: the T dimension is processed in four 512-chunks. Attention on
chunk ch is emitted with the (PE-only) QKV projection of chunk ch+1 and
the WO projection of chunk ch-1 round-robin-interleaved into its
(head-pair, k-tile) loop, so the PE array keeps streaming while ACT runs
the softmax exps — that keeps the HAM clock gate at 2.4 GHz. Heads are
processed in pairs living at partition offsets 0/64: their K=64 score
matmuls target disjoint row-halves of the PE array and run concurrently.
"""

import sys

sys.path.insert(0, "/opt/trn_rl_repo")

import numpy as np

import concourse.bass as bass  # noqa: F401  (AP helpers)
import concourse.mybir as mybir
import concourse.tile as tile
from concourse import bacc
from concourse.bass_utils import run_bass_kernel_spmd

F32 = mybir.dt.float32
F32R = mybir.dt.float32r
EXP = mybir.ActivationFunctionType.Exp

B, T, C, H = 4, 2048, 1024, 16
HD = 64  # head dim
HPC = 8  # heads per core
GF = HPC * HD  # 512 group features
CHUNK = 512
NCHUNK = T // CHUNK  # 4
KT_C = C // 128  # 8 k-tiles over C
SCALE = 1.0 / np.sqrt(HD)

_CACHE = {}


def _build():
    nc = bacc.Bacc(
        "TRN2", target_bir_lowering=False, debug=False, num_devices=8
    )
    xT = nc.dram_tensor("xT", [C, T], F32R, kind="ExternalInput")
    wqkvT = nc.dram_tensor("wqkvT", [C, 3 * GF], F32R, kind="ExternalInput")
    woT = nc.dram_tensor("woT", [GF, C], F32R, kind="ExternalInput")
    outT = nc.dram_tensor("outT", [C, T], F32, kind="ExternalOutput")

    xT_re = xT.ap().rearrange("(kt p) t -> p kt t", p=128)
    wq_re = wqkvT.ap().rearrange("(kt p) f -> p kt f", p=128)
    wo_re = woT.ap().rearrange("(kt p) f -> p kt f", p=128)

    with tile.TileContext(nc) as tc:
        with (
            tc.tile_pool(name="weights", bufs=1) as wp,
            tc.tile_pool(name="persist", bufs=1) as persist,
            tc.tile_pool(name="xin", bufs=2) as xp,
            tc.tile_pool(name="qt", bufs=2) as qtp,
            tc.tile_pool(name="pt", bufs=2) as ptp,
            tc.tile_pool(name="yt", bufs=3) as ytp,
            tc.tile_pool(name="small", bufs=2) as smp,
            tc.tile_pool(name="ost", bufs=2) as ostp,
            tc.tile_pool(name="proj", bufs=1, space="PSUM") as projp,
            tc.tile_pool(name="sps", bufs=2, space="PSUM") as spsp,
            tc.tile_pool(name="pops", bufs=3, space="PSUM") as pop,
        ):
            # wq split column-wise: [Q four 128-col tiles][K four][V one 512-col]
            wqq = [[wp.tile([128, 128], F32R, name=f"wqq{kt}_{g}") for g in range(4)]
                   for kt in range(KT_C)]
            wqk = [[wp.tile([128, 128], F32R, name=f"wqk{kt}_{g}") for g in range(4)]
                   for kt in range(KT_C)]
            wqv = [wp.tile([128, GF], F32R, name=f"wqv{kt}") for kt in range(KT_C)]
            wo_sb = wp.tile([128, GF // 128, C], F32R)

            ones32 = persist.tile([128, GF // 128 * HPC], F32)
            nc.vector.memset(ones32[:], 1.0)

            # per-chunk persistent K.T / V_aug
            KTc = [persist.tile([128, 4, CHUNK], F32R, name=f"KT{i}") for i in range(NCHUNK)]
            VAc = [persist.tile([128, 4, HPC, HD + 1], F32R, name=f"VA{i}") for i in range(NCHUNK)]
            for i in range(NCHUNK):
                nc.vector.tensor_copy(
                    VAc[i][:, :, :, HD : HD + 1],
                    ones32[:].rearrange("p (t h) -> p t h", t=4)[:, :, :, None],
                )

            xts = {}
            QTcs = {}
            ytcs = {}

            def load_x(ch):
                xts[ch] = [
                    xp.tile([128, CHUNK], F32R, tag="xt", bufs=11, name=f"xt{ch}_{kt}")
                    for kt in range(KT_C)
                ]
                for kt in range(KT_C):
                    nc.sync.dma_start(
                        xts[ch][kt][:],
                        xT_re[:, kt, ch * CHUNK : (ch + 1) * CHUNK],
                    )

            def qkv_thunks(ch, spread=False):
                """Projection chains for chunk ch (12 thunks)."""
                QTcs[ch] = qtp.tile([128, 4, CHUNK], F32R, tag="qtc", name=f"qtc{ch}")
                thunks = []
                seq = [0]

                def chain_psum():
                    if not spread:
                        return projp.tile([128, CHUNK], F32, tag="proj", name="ps")
                    i = seq[0]
                    seq[0] += 1
                    pool, tag = [
                        (projp, "proj"), (pop, "po"), (pop, "po"), (pop, "po"),
                        (spsp, "sps"), (spsp, "sps"),
                    ][i % 6]
                    return pool.tile([128, CHUNK], F32, tag=tag, name=f"pps{ch}_{i}")

                def qk_chain(m):
                    xt = xts[ch]
                    ps = chain_psum()
                    w = wqq[0][0]  # placeholder for loop binding
                    for kt in range(KT_C):
                        w = wqq[kt][m] if m < 4 else wqk[kt][m - 4]
                        nc.tensor.matmul(
                            ps[:],
                            w[:],
                            xt[kt][:],
                            start=(kt == 0),
                            stop=(kt == KT_C - 1),
                        )
                    dst = QTcs[ch][:, m] if m < 4 else KTc[ch][:, m - 4]
                    nc.vector.tensor_copy(dst, ps[:])

                def v_chain(t4):
                    xt = xts[ch]
                    ps = chain_psum()
                    for kt in range(KT_C):
                        nc.tensor.matmul(
                            ps[:],
                            xt[kt][:, t4 * 128 : (t4 + 1) * 128],
                            wqv[kt][:],
                            start=(kt == 0),
                            stop=(kt == KT_C - 1),
                        )
                    nc.vector.tensor_copy(
                        VAc[ch][:, t4, :, 0:HD],
                        ps[:].rearrange("p (h d) -> p h d", h=HPC),
                    )

                thunks.append(lambda: qk_chain(0))
                thunks.append(lambda: qk_chain(4))
                for t4 in range(4):
                    thunks.append(lambda t4=t4: v_chain(t4))
                for m in (1, 5, 2, 6, 3, 7):
                    thunks.append(lambda m=m: qk_chain(m))
                return thunks

            def wo_thunks(ch, pool=None, copy_eng=None):
                """Output projection chains for chunk ch (8 thunks)."""
                cs = slice(ch * CHUNK, (ch + 1) * CHUNK)
                wo_pool = pool or projp
                copy = copy_eng or nc.vector.tensor_copy

                def wo_chain(m):
                    ytc = ytcs[ch]
                    wps = wo_pool.tile([128, CHUNK], F32, tag="proj" if wo_pool is projp else "sps", name=f"wop{ch}_{m}")
                    for kt in range(GF // 128):
                        nc.tensor.matmul(
                            wps[:],
                            wo_sb[:, kt, m * 128 : (m + 1) * 128],
                            ytc[:, kt],
                            start=(kt == 0),
                            stop=(kt == GF // 128 - 1),
                        )
                    ot = ostp.tile([128, CHUNK], F32)
                    copy(ot[:], wps[:])
                    nc.sync.dma_start(outT[m * 128 : (m + 1) * 128, cs], ot[:])

                return [lambda m=m: wo_chain(m) for m in range(8)]

            def attention(qc, fill):
                """Attention for q-chunk qc, popping `fill` thunks along the way."""
                ytcs[qc] = ytp.tile([128, GF // 128, CHUNK], F32R, tag="ytc", name=f"ytc{qc}")
                ytc = ytcs[qc]
                QTc = QTcs[qc]
                nkt = (qc + 1) * 4
                nsteps = 4 * nkt
                stride = max(1, -(-nsteps // max(1, len(fill))))
                step = 0
                for pair in range(4):  # heads (2*pair, 2*pair+1)
                    pos = [
                        pop.tile([65, CHUNK], F32, tag="po", name=f"po{s}")
                        for s in range(2)
                    ]
                    for kt in range(nkt):
                        kc, lk = kt // 4, kt % 4
                        j = kt - 4 * qc
                        w = CHUNK if j < 0 else CHUNK - j * 128
                        q0 = CHUNK - w
                        sps2 = spsp.tile([128, 2 * CHUNK], F32, tag="sps")
                        for s in range(2):  # the two heads of the pair
                            off = s * 64
                            nc.tensor.matmul(
                                sps2[:, s * CHUNK : s * CHUNK + w],
                                KTc[kc][off : off + 64, pair, lk * 128 : (lk + 1) * 128],
                                QTc[off : off + 64, pair, q0:CHUNK],
                                start=True,
                                stop=True,
                            )
                        pt = ptp.tile([128, 2 * CHUNK], F32R)
                        nc.scalar.activation(
                            pt[:, 0 : CHUNK + w],
                            sps2[:, 0 : CHUNK + w],
                            EXP,
                            scale=SCALE,
                        )
                        if j >= 0:
                            for s in range(2):
                                nc.gpsimd.affine_select(
                                    out=pt[:, s * CHUNK : s * CHUNK + w],
                                    in_=pt[:, s * CHUNK : s * CHUNK + w],
                                    compare_op=mybir.AluOpType.is_ge,
                                    fill=0.0,
                                    base=0,
                                    pattern=[[1, w]],
                                    channel_multiplier=-1,
                                )
                        for s in range(2):
                            h = 2 * pair + s
                            nc.tensor.matmul(
                                pos[s][:, q0:CHUNK],
                                VAc[kc][:, lk, h],
                                pt[:, s * CHUNK : s * CHUNK + w],
                                start=(kt == 0),
                                stop=(kt == nkt - 1),
                            )
                        step += 1
                        if fill and step % stride == 0:
                            fill.pop(0)()
                    for s in range(2):
                        h = 2 * pair + s
                        off = s * 64
                        po = pos[s]
                        dn = smp.tile([1, CHUNK], F32, tag="nrm", bufs=2, name="dn")
                        nc.vector.tensor_copy(dn[:], po[64:65, :])
                        rc = smp.tile([1, CHUNK], F32, tag="nrm", bufs=2, name="rc")
                        nc.vector.reciprocal_approx_fast(rc[:], dn[:])
                        bc = smp.tile([64, CHUNK], F32, tag="nrm", bufs=2, name="bc")
                        nc.gpsimd.partition_broadcast(bc[:], rc[:])
                        nc.vector.tensor_mul(
                            ytc[off : off + 64, pair, :], po[0:64, :], bc[:]
                        )
                while fill:
                    fill.pop(0)()

            # prologue: x(0) first so QKV(0) starts ASAP; wo last (needed
            # only from the ch=2 window on)
            load_x(0)
            _dma_engs = [nc.gpsimd, nc.scalar, nc.sync]
            _dq = [0]

            def _dma(dst, srcslice):
                _dma_engs[_dq[0] % 3].dma_start(dst, srcslice)
                _dq[0] += 1

            # pair-0-critical first: Q col-group 0, K col-group 0, V block
            for kt in range(KT_C):
                _dma(wqq[kt][0][:], wq_re[:, kt, 0:128])
                _dma(wqk[kt][0][:], wq_re[:, kt, GF : GF + 128])
            for kt in range(KT_C):
                _dma(wqv[kt][:], wq_re[:, kt, 2 * GF : 3 * GF])
            for g in (1, 2, 3):
                for kt in range(KT_C):
                    _dma(wqq[kt][g][:], wq_re[:, kt, g * 128 : (g + 1) * 128])
                    _dma(wqk[kt][g][:], wq_re[:, kt, GF + g * 128 : GF + (g + 1) * 128])
            for kt in range(GF // 128):
                _dma(wo_sb[:, kt], wo_re[:, kt])
            for t in qkv_thunks(0, spread=True):
                t()
            for ch in range(NCHUNK):
                if ch + 1 < NCHUNK:
                    load_x(ch + 1)
                fill = []
                if ch + 1 < NCHUNK:
                    fill += qkv_thunks(ch + 1)
                if ch == NCHUNK - 1:
                    fill += wo_thunks(1) + wo_thunks(2)
                elif ch - 2 >= 0:
                    fill += wo_thunks(ch - 2)
                attention(ch, fill)
            for t in wo_thunks(NCHUNK - 1, pool=spsp, copy_eng=nc.scalar.copy):
                t()

    nc.compile()
    return nc


def _prep_inputs(x, wqkv, wo):
    """Per-core input maps: core c = (batch c // 2, head-group c % 2)."""
    x = np.asarray(x, dtype=np.float32)
    wqkv = np.asarray(wqkv, dtype=np.float32)
    wo = np.asarray(wo, dtype=np.float32)
    in_maps = []
    for c in range(8):
        b, g = c // 2, c % 2
        rows = np.r_[
            g * GF : (g + 1) * GF,
            C + g * GF : C + (g + 1) * GF,
            2 * C + g * GF : 2 * C + (g + 1) * GF,
        ]
        in_maps.append(
            {
                "xT": np.ascontiguousarray(x[b].T),
                "wqkvT": np.ascontiguousarray(wqkv[rows].T),
                "woT": np.ascontiguousarray(wo[:, g * GF : (g + 1) * GF].T),
            }
        )
    return in_maps


def _run(x, wqkv, wo, trace=False, trace_cores=None):
    if "nc" not in _CACHE:
        _CACHE["nc"] = _build()
    res = run_bass_kernel_spmd(
        _CACHE["nc"],
        _prep_inputs(x, wqkv, wo),
        core_ids=list(range(8)),
        trace=trace,
        trace_cores=trace_cores,
    )
    out = np.empty((B, T, C), dtype=np.float32)
    for b in range(B):
        out[b] = (res.results[2 * b]["outT"] + res.results[2 * b + 1]["outT"]).T
    return out, res


def kernel(x, wqkv, wo):
    out, _ = _run(x, wqkv, wo)
    return out


# revision 29
# speedup vs baseline: 1.0205x; 1.0147x over previous
"""Causal multi-head attention (B=4, T=2048, C=1024, H=16) on 8 TRN2 cores.

Sharding: core c = (batch b = c // 2, head-group g = c % 2). Each core runs
QKV projection + attention + its half of the output projection for one
batch and 8 heads (Megatron-style column-parallel wqkv / row-parallel wo);
the two partial wo products per batch are summed on the host during
unsharding.

Device layouts (per core):
  xT    [C, T]        x[b] transposed (C on partitions for QKV matmuls)
  wqkvT [C, 3*512]    head-group slice of wqkv, transposed; cols [Q|K|V]
  woT   [512, C]      wo columns for this head-group, transposed
  outT  [C, T]        partial (x @ wqkv.T -> attn -> @ wo.T).T

All matmuls run in float32r (tf32-like: ~1.5e-4 rel err, ~3.3x faster
than fp32 on the PE). Scores are computed transposed (k on partitions) so
softmax P.T feeds the AV matmul directly with no transposes; the softmax
denominator rides along as a 65th row of V (column of ones), and the
causal mask is an affine_select on the diagonal score tiles. exp() is
applied without max-subtraction: scores are ~N(0,1), far inside fp32 exp
range.

Scheduling: the T dimension is processed in four 512-chunks. Attention on
chunk ch is emitted with the (PE-only) QKV projection of chunk ch+1 and
the WO projection of chunk ch-1 round-robin-interleaved into its
(head-pair, k-tile) loop, so the PE array keeps streaming while ACT runs
the softmax exps — that keeps the HAM clock gate at 2.4 GHz. Heads are
processed in pairs living at partition offsets 0/64: their K=64 score
matmuls target disjoint row-halves of the PE array and run concurrently.
"""

import sys

sys.path.insert(0, "/opt/trn_rl_repo")

import numpy as np

import concourse.bass as bass  # noqa: F401  (AP helpers)
import concourse.mybir as mybir
import concourse.tile as tile
from concourse import bacc
from concourse.bass_utils import run_bass_kernel_spmd

F32 = mybir.dt.float32
F32R = mybir.dt.float32r
EXP = mybir.ActivationFunctionType.Exp

B, T, C, H = 4, 2048, 1024, 16
HD = 64  # head dim
HPC = 8  # heads per core
GF = HPC * HD  # 512 group features
CHUNK = 512
NCHUNK = T // CHUNK  # 4
KT_C = C // 128  # 8 k-tiles over C
SCALE = 1.0 / np.sqrt(HD)

_CACHE = {}


def _build():
    nc = bacc.Bacc(
        "TRN2", target_bir_lowering=False, debug=False, num_devices=8
    )
    xT = nc.dram_tensor("xT", [C, T], F32R, kind="ExternalInput")
    wqkvT = nc.dram_tensor("wqkvT", [C, 3 * GF], F32R, kind="ExternalInput")
    woT = nc.dram_tensor("woT", [GF, C], F32R, kind="ExternalInput")
    outT = nc.dram_tensor("outT", [C, T], F32, kind="ExternalOutput")

    xT_re = xT.ap().rearrange("(kt p) t -> p kt t", p=128)
    wq_re = wqkvT.ap().rearrange("(kt p) f -> p kt f", p=128)
    wo_re = woT.ap().rearrange("(kt p) f -> p kt f", p=128)

    with tile.TileContext(nc) as tc:
        with (
            tc.tile_pool(name="weights", bufs=1) as wp,
            tc.tile_pool(name="persist", bufs=1) as persist,
            tc.tile_pool(name="xin", bufs=2) as xp,
            tc.tile_pool(name="qt", bufs=2) as qtp,
            tc.tile_pool(name="pt", bufs=2) as ptp,
            tc.tile_pool(name="yt", bufs=3) as ytp,
            tc.tile_pool(name="small", bufs=2) as smp,
            tc.tile_pool(name="ost", bufs=2) as ostp,
            tc.tile_pool(name="proj", bufs=1, space="PSUM") as projp,
            tc.tile_pool(name="sps", bufs=2, space="PSUM") as spsp,
            tc.tile_pool(name="pops", bufs=3, space="PSUM") as pop,
        ):
            # wq split column-wise: [Q four 128-col tiles][K four][V one 512-col]
            wqq = [[wp.tile([128, 128], F32R, name=f"wqq{kt}_{g}") for g in range(4)]
                   for kt in range(KT_C)]
            wqk = [[wp.tile([128, 128], F32R, name=f"wqk{kt}_{g}") for g in range(4)]
                   for kt in range(KT_C)]
            wqv = [wp.tile([128, GF], F32R, name=f"wqv{kt}") for kt in range(KT_C)]
            wo_sb = wp.tile([128, GF // 128, C], F32R)

            ones32 = persist.tile([128, GF // 128 * HPC], F32)
            nc.vector.memset(ones32[:], 1.0)

            # per-chunk persistent K.T / V_aug
            KTc = [persist.tile([128, 4, CHUNK], F32R, name=f"KT{i}") for i in range(NCHUNK)]
            VAc = [persist.tile([128, 4, HPC, HD + 1], F32R, name=f"VA{i}") for i in range(NCHUNK)]
            for i in range(NCHUNK):
                nc.vector.tensor_copy(
                    VAc[i][:, :, :, HD : HD + 1],
                    ones32[:].rearrange("p (t h) -> p t h", t=4)[:, :, :, None],
                )

            xts = {}
            QTcs = {}
            ytcs = {}

            def load_x(ch):
                xts[ch] = [
                    xp.tile([128, CHUNK], F32R, tag="xt", bufs=11, name=f"xt{ch}_{kt}")
                    for kt in range(KT_C)
                ]
                for kt in range(KT_C):
                    nc.sync.dma_start(
                        xts[ch][kt][:],
                        xT_re[:, kt, ch * CHUNK : (ch + 1) * CHUNK],
                    )

            def qkv_thunks(ch, spread=False):
                """Projection chains for chunk ch (12 thunks)."""
                QTcs[ch] = qtp.tile([128, 4, CHUNK], F32R, tag="qtc", name=f"qtc{ch}")
                thunks = []
                seq = [0]

                def chain_psum():
                    if not spread:
                        return projp.tile([128, CHUNK], F32, tag="proj", name="ps")
                    i = seq[0]
                    seq[0] += 1
                    pool, tag = [
                        (projp, "proj"), (pop, "po"), (pop, "po"), (pop, "po"),
                        (spsp, "sps"), (spsp, "sps"),
                    ][i % 6]
                    return pool.tile([128, CHUNK], F32, tag=tag, name=f"pps{ch}_{i}")

                def qk_chain(m):
                    xt = xts[ch]
                    ps = chain_psum()
                    w = wqq[0][0]  # placeholder for loop binding
                    for kt in range(KT_C):
                        w = wqq[kt][m] if m < 4 else wqk[kt][m - 4]
                        nc.tensor.matmul(
                            ps[:],
                            w[:],
                            xt[kt][:],
                            start=(kt == 0),
                            stop=(kt == KT_C - 1),
                        )
                    dst = QTcs[ch][:, m] if m < 4 else KTc[ch][:, m - 4]
                    nc.vector.tensor_copy(dst, ps[:])

                def v_chain(t4):
                    xt = xts[ch]
                    ps = chain_psum()
                    for kt in range(KT_C):
                        nc.tensor.matmul(
                            ps[:],
                            xt[kt][:, t4 * 128 : (t4 + 1) * 128],
                            wqv[kt][:],
                            start=(kt == 0),
                            stop=(kt == KT_C - 1),
                        )
                    nc.vector.tensor_copy(
                        VAc[ch][:, t4, :, 0:HD],
                        ps[:].rearrange("p (h d) -> p h d", h=HPC),
                    )

                thunks.append(lambda: qk_chain(0))
                thunks.append(lambda: qk_chain(4))
                for t4 in range(4):
                    thunks.append(lambda t4=t4: v_chain(t4))
                for m in (1, 5, 2, 6, 3, 7):
                    thunks.append(lambda m=m: qk_chain(m))
                return thunks

            def wo_thunks(ch, pool=None, copy_eng=None):
                """Output projection chains for chunk ch (8 thunks)."""
                cs = slice(ch * CHUNK, (ch + 1) * CHUNK)
                wo_pool = pool or projp
                copy = copy_eng or nc.vector.tensor_copy

                def wo_chain(m):
                    ytc = ytcs[ch]
                    wps = wo_pool.tile([128, CHUNK], F32, tag="proj" if wo_pool is projp else "sps", name=f"wop{ch}_{m}")
                    for kt in range(GF // 128):
                        nc.tensor.matmul(
                            wps[:],
                            wo_sb[:, kt, m * 128 : (m + 1) * 128],
                            ytc[:, kt],
                            start=(kt == 0),
                            stop=(kt == GF // 128 - 1),
                        )
                    ot = ostp.tile([128, CHUNK], F32)
                    copy(ot[:], wps[:])
                    nc.sync.dma_start(outT[m * 128 : (m + 1) * 128, cs], ot[:])

                return [lambda m=m: wo_chain(m) for m in range(8)]

            def attention(qc, fill):
                """Attention for q-chunk qc, popping `fill` thunks along the way."""
                ytcs[qc] = ytp.tile([128, GF // 128, CHUNK], F32R, tag="ytc", name=f"ytc{qc}")
                ytc = ytcs[qc]
                QTc = QTcs[qc]
                nkt = (qc + 1) * 4
                nsteps = 4 * nkt
                stride = max(1, -(-nsteps // max(1, len(fill))))
                step = 0
                for pair in range(4):  # heads (2*pair, 2*pair+1)
                    pos = [
                        pop.tile([65, CHUNK], F32, tag="po", name=f"po{s}")
                        for s in range(2)
                    ]
                    for kt in range(nkt):
                        kc, lk = kt // 4, kt % 4
                        j = kt - 4 * qc
                        w = CHUNK if j < 0 else CHUNK - j * 128
                        q0 = CHUNK - w
                        sps2 = spsp.tile([128, 2 * CHUNK], F32, tag="sps")
                        for s in range(2):  # the two heads of the pair
                            off = s * 64
                            nc.tensor.matmul(
                                sps2[:, s * CHUNK : s * CHUNK + w],
                                KTc[kc][off : off + 64, pair, lk * 128 : (lk + 1) * 128],
                                QTc[off : off + 64, pair, q0:CHUNK],
                                start=True,
                                stop=True,
                            )
                        pt = ptp.tile([128, 2 * CHUNK], F32R)
                        nc.scalar.activation(
                            pt[:, 0 : CHUNK + w],
                            sps2[:, 0 : CHUNK + w],
                            EXP,
                            scale=SCALE,
                        )
                        if j >= 0:
                            for s in range(2):
                                nc.gpsimd.affine_select(
                                    out=pt[:, s * CHUNK : s * CHUNK + w],
                                    in_=pt[:, s * CHUNK : s * CHUNK + w],
                                    compare_op=mybir.AluOpType.is_ge,
                                    fill=0.0,
                                    base=0,
                                    pattern=[[1, w]],
                                    channel_multiplier=-1,
                                )
                        for s in range(2):
                            h = 2 * pair + s
                            nc.tensor.matmul(
                                pos[s][:, q0:CHUNK],
                                VAc[kc][:, lk, h],
                                pt[:, s * CHUNK : s * CHUNK + w],
                                start=(kt == 0),
                                stop=(kt == nkt - 1),
                            )
                        step += 1
                        if fill and step % stride == 0:
                            fill.pop(0)()
                    for s in range(2):
                        h = 2 * pair + s
                        off = s * 64
                        po = pos[s]
                        dn = smp.tile([1, CHUNK], F32, tag="nrm", bufs=2, name="dn")
                        nc.vector.tensor_copy(dn[:], po[64:65, :])
                        rc = smp.tile([1, CHUNK], F32, tag="nrm", bufs=2, name="rc")
                        nc.vector.reciprocal_approx_fast(rc[:], dn[:])
                        bc = smp.tile([64, CHUNK], F32, tag="nrm", bufs=2, name="bc")
                        nc.gpsimd.partition_broadcast(bc[:], rc[:])
                        nc.vector.tensor_mul(
                            ytc[off : off + 64, pair, :], po[0:64, :], bc[:]
                        )
                while fill:
                    fill.pop(0)()

            # prologue: x(0) first so QKV(0) starts ASAP; wo last (needed
            # only from the ch=2 window on)
            load_x(0)  # x(0) rides the sync queue alone
            _dma_engs = [nc.gpsimd, nc.scalar, nc.sync]
            _dq = [0]

            def _dma(dst, srcslice, n=3):
                _dma_engs[_dq[0] % n].dma_start(dst, srcslice)
                _dq[0] += 1

            # pair-0-critical first, on gpsimd+scalar only (parallel to x(0))
            for kt in range(KT_C):
                _dma(wqq[kt][0][:], wq_re[:, kt, 0:128], n=2)
                _dma(wqk[kt][0][:], wq_re[:, kt, GF : GF + 128], n=2)
            for kt in range(KT_C):
                _dma(wqv[kt][:], wq_re[:, kt, 2 * GF : 3 * GF], n=2)
            for g in (1, 2, 3):
                for kt in range(KT_C):
                    _dma(wqq[kt][g][:], wq_re[:, kt, g * 128 : (g + 1) * 128])
                    _dma(wqk[kt][g][:], wq_re[:, kt, GF + g * 128 : GF + (g + 1) * 128])
            for kt in range(GF // 128):
                _dma(wo_sb[:, kt], wo_re[:, kt])
            for t in qkv_thunks(0, spread=True):
                t()
            for ch in range(NCHUNK):
                if ch + 1 < NCHUNK:
                    load_x(ch + 1)
                fill = []
                if ch + 1 < NCHUNK:
                    fill += qkv_thunks(ch + 1)
                if ch == NCHUNK - 1:
                    fill += wo_thunks(1) + wo_thunks(2)
                elif ch - 2 >= 0:
                    fill += wo_thunks(ch - 2)
                attention(ch, fill)
            for t in wo_thunks(NCHUNK - 1, pool=spsp, copy_eng=nc.scalar.copy):
                t()

    nc.compile()
    return nc


def _prep_inputs(x, wqkv, wo):
    """Per-core input maps: core c = (batch c // 2, head-group c % 2)."""
    x = np.asarray(x, dtype=np.float32)
    wqkv = np.asarray(wqkv, dtype=np.float32)
    wo = np.asarray(wo, dtype=np.float32)
    in_maps = []
    for c in range(8):
        b, g = c // 2, c % 2
        rows = np.r_[
            g * GF : (g + 1) * GF,
            C + g * GF : C + (g + 1) * GF,
            2 * C + g * GF : 2 * C + (g + 1) * GF,
        ]
        in_maps.append(
            {
                "xT": np.ascontiguousarray(x[b].T),
                "wqkvT": np.ascontiguousarray(wqkv[rows].T),
                "woT": np.ascontiguousarray(wo[:, g * GF : (g + 1) * GF].T),
            }
        )
    return in_maps


def _run(x, wqkv, wo, trace=False, trace_cores=None):
    if "nc" not in _CACHE:
        _CACHE["nc"] = _build()
    res = run_bass_kernel_spmd(
        _CACHE["nc"],
        _prep_inputs(x, wqkv, wo),
        core_ids=list(range(8)),
        trace=trace,
        trace_cores=trace_cores,
    )
    out = np.empty((B, T, C), dtype=np.float32)
    for b in range(B):
        out[b] = (res.results[2 * b]["outT"] + res.results[2 * b + 1]["outT"]).T
    return out, res


def kernel(x, wqkv, wo):
    out, _ = _run(x, wqkv, wo)
    return out
